# revision 42
# baseline (speedup 1.0000x reference)
"""Trainium2 Bass kernel for nn_CustomClassifier_56307021251047 (compact bilinear pooling).

Math reformulation (validated against reference in numpy):
  count-sketch + rfft fold into dense matmuls with host-precomputed trig
  matrices:  A_s[b,f] = sum_i s_i x[b,i] e^{-2pi i h_i f / D}  ==  x @ C_s
  (C_s[i,f] built from (h_s, s_s) on host; the Nyquist bin f=D/2, which is
  real like bin 0, is packed into bin 0's imaginary slot so the spectrum is
  exactly NF = D/2 = 4096 bins = 8 cores x 512).
  Y = A_1 * A_2 (complex pointwise; bin 0 uses (re*re, im*im) instead).
  cbp = Yre @ ICre + Yim @ ICim  (inverse rfft as matmul, Nyquist row fixed).
  out = cbp @ W.T + b.

Sharding over 8 cores: spectrum dim f (4096 = 8 x 512) for stages 1-2,
AllReduce of the partial cbp^T [8192, 256], then classes (14951 -> 8 chunks)
for the Linear.
"""

import os
import sys

sys.path.insert(0, "/opt/trn_rl_repo")

import numpy as np

B, F_IN, D, N_CLASSES = 256, 2048, 8192, 14951
N_CORES = 8
NF = D // 2              # 4096 one-sided spectrum bins (Nyquist packed in bin0.im)
FC = NF // N_CORES       # 512 bins per core
FT = FC // 128           # 4 f-tiles per core
KX = F_IN // 128         # 16 k-tiles over input features
NT = D // 128            # 64 n-tiles of cbp
CPAD = 1872              # per-core padded class count (8*1872 >= 14951)
CW = [512, 512, 512, 336]       # class chunks within CPAD
C0 = [0, 512, 1024, 1536]
# class ranges per core: 7 x 1869 + 1868
_CLS_SIZES = [1869] * 7 + [1868]
_CLS_OFF = np.cumsum([0] + _CLS_SIZES).tolist()

# dtype knobs (env-overridable for experiments)
_DT_S1 = os.environ.get("K_DT_S1", "bfloat16")  # stage-1 matmul (cpack & xT)
_DT_S2 = os.environ.get("K_DT_S2", "bfloat16")  # stage-2 matmul (IC & Y)
_DT_W = os.environ.get("K_DT_W", "bfloat16")    # stage-3 matmul (W & cbp lhsT)


LAST_RESULTS = None  # BassKernelResults of the final launch, for test.py


def _np_dt(name):
    if name == "bfloat16":
        import ml_dtypes

        return ml_dtypes.bfloat16
    return np.dtype(name)


def _mybir_dt(name):
    import concourse.mybir as mybir

    return {"float32": mybir.dt.float32, "bfloat16": mybir.dt.bfloat16}[name]


def _build_program():
    import concourse.bacc as bacc
    import concourse.tile as tile
    import concourse.mybir as mybir

    f32 = mybir.dt.float32
    dt1 = _mybir_dt(_DT_S1)
    dt2 = _mybir_dt(_DT_S2)
    dtw = _mybir_dt(_DT_W)

    nc = bacc.Bacc("TRN2", target_bir_lowering=False, debug=False,
                   num_devices=N_CORES)

    xt = nc.dram_tensor("xt", [F_IN, B], dt1, kind="ExternalInput")
    cpack = nc.dram_tensor("cpack", [4, 128, 4, 4, FC], dt1, kind="ExternalInput")
    icpack = nc.dram_tensor("icpack", [8, 128, 64, 128], dt2, kind="ExternalInput")
    wpack = nc.dram_tensor("wpack", [NT + 1, 128, CPAD], dtw, kind="ExternalInput")
    fmask = nc.dram_tensor("fmask", [128, FT, 3], f32, kind="ExternalInput")
    out = nc.dram_tensor("out", [B, CPAD], f32, kind="ExternalOutput")

    rg = [list(range(N_CORES))]

    with tile.TileContext(nc) as tc:
        with (
            tc.tile_pool(name="const", bufs=1) as const,
            tc.tile_pool(name="dram", bufs=1, space="DRAM") as dram,
        ):
            # AllGather buffers: Y spectra (stage 1 -> 2) and cbp slabs (2 -> 3)
            ag1_ins = [
                dram.tile([128, FT, B], dt2, tag=f"ag1i{h}", name=f"ag1_in_{h}")
                for h in range(2)
            ]
            ag1_outs = [
                dram.tile([N_CORES, 128, FT, B], dt2, tag=f"ag1o{h}",
                          name=f"ag1_out_{h}", addr_space="Shared")
                for h in range(2)
            ]
            ag2_ins = [
                dram.tile([128, 4, B], dtw, tag=f"ag2i{j}", name=f"ag2_in_{j}")
                for j in range(2)
            ]
            ag2_outs = [
                dram.tile([N_CORES, 128, 4, B], dtw, tag=f"ag2o{j}",
                          name=f"ag2_out_{j}", addr_space="Shared")
                for j in range(2)
            ]

            fm = const.tile([128, FT, 3], f32)
            nc.sync.dma_start(out=fm[:], in_=fmask[:])

            # PE warmup: keep TensorE busy through the initial DMA ramp so
            # the HAM clock gate opens (1.2 -> 2.4 GHz) before stage 1.
            n_warm = int(os.environ.get("K_WARM", "48"))
            if n_warm:
                with tc.tile_pool(name="warm", bufs=1, space="PSUM") as warmp:
                    wsrc = const.tile([128, B], _mybir_dt(_DT_S1), name="wsrc")
                    nc.vector.memset(wsrc[:], 0.0)
                    wps = warmp.tile([128, B], f32, name="warm_ps")
                    for _ in range(n_warm):
                        nc.tensor.matmul(wps[:], wsrc[:, 0:128], wsrc[:],
                                         start=True, stop=True)
                    nc.vector.tensor_copy(out=wsrc[0:1, 0:8], in_=wps[0:1, 0:8])

            # ---------------- stage 1 + pointwise ----------------
            with (
                tc.tile_pool(name="xpool", bufs=1) as xpool,
                tc.tile_pool(name="apool", bufs=1) as apool,
                tc.tile_pool(name="ypool", bufs=1) as ypool,
                tc.tile_pool(name="cpool", bufs=1) as cpool,
                tc.tile_pool(name="tpool", bufs=1) as tpool,
            ):
                xt_sb = xpool.tile([128, KX, B], dt1)
                nc.sync.dma_start(
                    out=xt_sb[:], in_=xt.rearrange("(k p) b -> p k b", p=128)
                )
                # resident C matrices: [p, k, j, f] (8.4 MB bf16), 4 big DMAs
                cpk = cpool.tile([128, KX, 4, FC], dt1)
                for g in range(4):
                    nc.sync.dma_start(
                        out=cpk[:, 4 * g:4 * g + 4, :, :], in_=cpack[g]
                    )

                # stage 1: A_jT [FC, B] for j in (A1re, A1im, A2re, A2im),
                # halves grouped by f-tile so pointwise overlaps half 2.
                # a_sb tile index = j*FT + mt; y_sb index = rm*FT + ft
                a_sb = apool.tile([128, 4 * FT, B], f32)
                y_sb = ypool.tile([128, 2 * FT, B], dt2)
                with tc.tile_pool(name="ps1", bufs=1, space="PSUM") as ps1pool:
                    for h in range(2):  # f-tile halves: mt in {2h, 2h+1}
                        psums = [
                            ps1pool.tile([128, B], f32, tag=f"ps1_{j}_{mtl}",
                                         name=f"ps1_{h}_{j}_{mtl}")
                            for j in range(4)
                            for mtl in range(2)
                        ]
                        for k in range(KX):
                            for j in range(4):
                                for mtl in range(2):
                                    mt = 2 * h + mtl
                                    nc.tensor.matmul(
                                        psums[j * 2 + mtl][:],
                                        cpk[:, k, j, mt * 128:(mt + 1) * 128],
                                        xt_sb[:, k, :],
                                        start=(k == 0),
                                        stop=(k == KX - 1),
                                    )
                        for j in range(4):
                            for mtl in range(2):
                                mt = 2 * h + mtl
                                nc.vector.tensor_copy(
                                    out=a_sb[:, j * FT + mt, :],
                                    in_=psums[j * 2 + mtl][:],
                                )
                        # pointwise for ft in this half: Y = A1*A2 complex,
                        # with packed-bin fix via masks
                        for mtl in range(2):
                            ft = 2 * h + mtl
                            r1 = a_sb[:, 0 * FT + ft, :]
                            i1 = a_sb[:, 1 * FT + ft, :]
                            r2 = a_sb[:, 2 * FT + ft, :]
                            i2 = a_sb[:, 3 * FT + ft, :]
                            cm = fm[:, ft, 0:1]
                            ncm = fm[:, ft, 1:2]
                            mk = fm[:, ft, 2:3]
                            tt = tpool.tile([128, 4, B], f32, tag="ptmp",
                                            name=f"ptmp_{ft}")
                            T, U, V, W2 = (tt[:, i, :] for i in range(4))
                            nc.vector.tensor_mul(out=T, in0=i1, in1=i2)
                            nc.vector.tensor_mul(out=U, in0=r1, in1=r2)
                            # Yre = (T * ncm) + U
                            nc.vector.scalar_tensor_tensor(
                                out=y_sb[:, ft * 2, :], in0=T, scalar=ncm,
                                in1=U, op0=mybir.AluOpType.mult,
                                op1=mybir.AluOpType.add,
                            )
                            nc.vector.tensor_mul(out=V, in0=r1, in1=i2)
                            nc.vector.tensor_mul(out=W2, in0=i1, in1=r2)
                            nc.vector.tensor_add(out=V, in0=V, in1=W2)
                            nc.vector.tensor_scalar_mul(T, T, mk)
                            # Yim = (V * cm) + (T * mk)
                            nc.vector.scalar_tensor_tensor(
                                out=y_sb[:, ft * 2 + 1, :], in0=V, scalar=cm,
                                in1=T, op0=mybir.AluOpType.mult,
                                op1=mybir.AluOpType.add,
                            )
                        # ship this half's Y (tiles 4h..4h+4) and gather
                        nc.sync.dma_start(
                            out=ag1_ins[h].opt(),
                            in_=y_sb[:, 4 * h:4 * h + 4, :],
                        )
                        nc.gpsimd.collective_compute(
                            "AllGather", mybir.AluOpType.bypass,
                            replica_groups=rg,
                            ins=[ag1_ins[h].opt()], outs=[ag1_outs[h].opt()],
                        )

            # ---------------- stage 2 (n-sharded over cores) ----------------
            # cbpT[n_m, b] = sum over ALL f of IC[f, n_m] * YT[f, b],
            # n_m = this core's 1024-wide slab; kstep = c*8 + rm*4 + ftl
            with (
                tc.tile_pool(name="yfpool", bufs=1) as yfpool,
                tc.tile_pool(name="icpool", bufs=3) as icpool,
                tc.tile_pool(name="cbspool", bufs=2) as cbspool,
                tc.tile_pool(name="ps2", bufs=4, space="PSUM") as ps2pool,
            ):
                # y_full slot ks = h*32 + c*4 + (ftl2*2 + rm)
                y_full = yfpool.tile([128, 8 * N_CORES, B], dt2)
                for h in range(2):
                    for c in range(N_CORES):
                        nc.sync.dma_start(
                            out=y_full[:, h * 32 + c * 4:h * 32 + c * 4 + 4, :],
                            in_=ag1_outs[h][c],
                        )
                for nt in range(8):
                    ic_tile = icpool.tile([128, 64, 128], dt2, tag="ictile",
                                          name=f"ic_{nt}")
                    nc.sync.dma_start(out=ic_tile[:], in_=icpack[nt])
                    half = nt // 4
                    if nt % 4 == 0:
                        cbs = cbspool.tile([128, 4, B], dtw, tag="cbs",
                                           name=f"cbs_{half}")
                    ps = ps2pool.tile([128, B], f32, tag="ps2",
                                      name=f"ps2_{nt}")
                    for ks in range(64):
                        nc.tensor.matmul(
                            ps[:],
                            ic_tile[:, ks, :],
                            y_full[:, ks, :],
                            start=(ks == 0),
                            stop=(ks == 63),
                        )
                    nc.vector.tensor_copy(out=cbs[:, nt % 4, :], in_=ps[:])
                    if nt % 4 == 3:
                        nc.sync.dma_start(out=ag2_ins[half].opt(), in_=cbs[:])
                        nc.gpsimd.collective_compute(
                            "AllGather", mybir.AluOpType.bypass,
                            replica_groups=rg,
                            ins=[ag2_ins[half].opt()],
                            outs=[ag2_outs[half].opt()],
                        )

            # ---------------- stage 3 ----------------
            with (
                tc.tile_pool(name="cbpool", bufs=3) as cbpool,
                tc.tile_pool(name="wpool", bufs=6) as wpool,
                tc.tile_pool(name="opool", bufs=1) as opool,
                tc.tile_pool(name="ps3", bufs=1, space="PSUM") as ps3pool,
            ):
                kones = const.tile([128, B], dtw)
                nc.vector.memset(kones[:], 0.0)
                nc.vector.memset(kones[0:1, :], 1.0)

                psums3 = [
                    ps3pool.tile([128, CW[cc]], f32, tag=f"ps3_{cc}_{bh}",
                                 name=f"ps3_{cc}_{bh}")
                    for cc in range(4)
                    for bh in range(2)
                ]
                # bias step first (k = NT): runs before any AR data lands
                w_tile = wpool.tile([128, CPAD], dtw, tag="wt", name="wt_bias")
                nc.sync.dma_start(out=w_tile[:], in_=wpack[NT])
                for cc in range(4):
                    for bh in range(2):
                        nc.tensor.matmul(
                            psums3[cc * 2 + bh][:],
                            kones[:, bh * 128:(bh + 1) * 128],
                            w_tile[:, C0[cc]:C0[cc] + CW[cc]],
                            start=True,
                            stop=False,
                        )
                for j in range(2):
                    for c in range(N_CORES):
                        cb_tile = cbpool.tile([128, 4, B], dtw, tag="cbt",
                                              name=f"cbt_{j}_{c}")
                        nc.sync.dma_start(out=cb_tile[:], in_=ag2_outs[j][c])
                        for i in range(4):
                            k = 8 * c + 4 * j + i
                            w_tile = wpool.tile([128, CPAD], dtw, tag="wt",
                                                name=f"wt_{k}")
                            nc.sync.dma_start(out=w_tile[:], in_=wpack[k])
                            last = (j == 1 and c == N_CORES - 1 and i == 3)
                            for cc in range(4):
                                for bh in range(2):
                                    nc.tensor.matmul(
                                        psums3[cc * 2 + bh][:],
                                        cb_tile[:, i, bh * 128:(bh + 1) * 128],
                                        w_tile[:, C0[cc]:C0[cc] + CW[cc]],
                                        start=False,
                                        stop=last,
                                    )

                out_sb = opool.tile([128, 2, CPAD], f32)
                for cc in range(4):
                    for bh in range(2):
                        nc.vector.tensor_copy(
                            out=out_sb[:, bh, C0[cc]:C0[cc] + CW[cc]],
                            in_=psums3[cc * 2 + bh][:],
                        )
                nc.sync.dma_start(
                    out=out.rearrange("(h p) c -> p h c", p=128), in_=out_sb[:]
                )

    nc.compile()
    return nc


def _precompute_inputs(x, h1, h2, s1, s2, W, b):
    """Host-side packing of weights/constants into per-core input maps."""
    dt1 = _np_dt(_DT_S1)
    dt2 = _np_dt(_DT_S2)
    dtw = _np_dt(_DT_W)

    h1 = np.asarray(h1).astype(np.int64)
    h2 = np.asarray(h2).astype(np.int64)
    s1 = np.asarray(s1).astype(np.float64)
    s2 = np.asarray(s2).astype(np.float64)
    x = np.asarray(x, dtype=np.float32)
    W = np.asarray(W, dtype=np.float32)
    b = np.asarray(b, dtype=np.float32)

    xt_np = np.ascontiguousarray(x.T).astype(dt1)  # [F_IN, B]

    f = np.arange(NF, dtype=np.int64)

    def build_C(h, s):
        ang = 2.0 * np.pi * ((np.outer(h, f) % D).astype(np.float64)) / D
        Cre = s[:, None] * np.cos(ang)
        Cim = -s[:, None] * np.sin(ang)
        Cim[:, 0] = s * np.where(h % 2 == 0, 1.0, -1.0)  # Nyquist real part
        return Cre.astype(np.float32), Cim.astype(np.float32)

    C1re, C1im = build_C(h1, s1)
    C2re, C2im = build_C(h2, s2)

    n = np.arange(D, dtype=np.int64)
    ang = 2.0 * np.pi * ((np.outer(f, n) % D).astype(np.float64)) / D
    w_f = np.full((NF, 1), 2.0)
    w_f[0] = 1.0
    ICre = (w_f * np.cos(ang)) / D
    ICim = (-w_f * np.sin(ang)) / D
    ICim[0, :] = np.where(n % 2 == 0, 1.0, -1.0) / D  # packed Nyquist row
    ICre = ICre.astype(np.float32)
    ICim = ICim.astype(np.float32)

    in_maps = []
    for m in range(N_CORES):
        fsl = slice(m * FC, (m + 1) * FC)
        # cpack [4, 128, 4, 4, FC]: [g][p][kk][j][f], k = 4g + kk
        cp = np.empty((4, 128, 4, 4, FC), np.float32)
        for j_i, cmat in enumerate((C1re, C1im, C2re, C2im)):
            cj = cmat[:, fsl].reshape(4, 4, 128, FC)  # [g, kk, p, f]
            cp[:, :, :, j_i, :] = cj.transpose(0, 2, 1, 3)
        # icpack [8, 128, 64, 128]: [nt][f_in_tile][kstep][nn] where
        # kstep = h*32 + c*4 + (ftl%2)*2 + rm  (h = ftl//2),
        # f = c*512 + ftl*128 + p, n = m*1024 + nt*128 + nn (this core's slab)
        icp = np.empty((8, 128, 64, 128), np.float32)
        nsl = slice(m * 8 * 128, (m + 1) * 8 * 128)
        for rm, icm in enumerate((ICre, ICim)):
            # [c, ftl, p, nt, nn]
            blk = icm[:, nsl].reshape(N_CORES, FT, 128, 8, 128)
            for c in range(N_CORES):
                for ftl in range(FT):
                    ks = (ftl // 2) * 32 + c * 4 + (ftl % 2) * 2 + rm
                    icp[:, :, ks, :] = blk[c, ftl].transpose(1, 0, 2)
        # wpack [65, 128, CPAD]
        c_lo, c_hi = _CLS_OFF[m], _CLS_OFF[m + 1]
        sz = c_hi - c_lo
        wp = np.zeros((NT + 1, 128, CPAD), np.float32)
        wp[:NT, :, :sz] = W[c_lo:c_hi].T.reshape(NT, 128, sz)
        wp[NT, 0, :sz] = b[c_lo:c_hi]
        # fmask [128, FT, 3]: cm, negcm, mask
        fmsk = np.zeros((128, FT, 3), np.float32)
        fmsk[:, :, 0] = 1.0
        fmsk[:, :, 1] = -1.0
        if m == 0:
            fmsk[0, 0, 0] = 0.0
            fmsk[0, 0, 1] = 0.0
            fmsk[0, 0, 2] = 1.0
        in_maps.append(
            {
                "xt": xt_np,
                "cpack": cp.astype(dt1),
                "icpack": icp.astype(dt2),
                "wpack": wp.astype(dtw),
                "fmask": fmsk,
            }
        )
    return in_maps


_PROGRAM = None


def _ensure_ntff_hook():
    """bass_utils' axon trace path imports antenv.axon_hooks, which this
    container's antenv stub lacks. Provide it, backed by the ctypes NTFF
    profiler from trn_boot, so BASS_TRACE=1 profiling works."""
    import types

    try:
        import antenv.axon_hooks  # noqa: F401

        return
    except ImportError:
        pass
    try:
        import antenv
    except ImportError:
        return
    state = {"hook": None, "tried": False}

    def set_axon_ntff_profile_hook(h):
        state["hook"] = h
        state["tried"] = True

    def get_axon_ntff_profile_hook():
        if not state["tried"]:
            state["tried"] = True
            try:
                from trn_agent_boot.trn_boot import _ntff_profile_via_ctypes

                state["hook"] = _ntff_profile_via_ctypes("/opt/axon/libaxon_pjrt.so")
            except Exception:
                state["hook"] = None
        return state["hook"]

    mod = types.ModuleType("antenv.axon_hooks")
    mod.set_axon_ntff_profile_hook = set_axon_ntff_profile_hook
    mod.get_axon_ntff_profile_hook = get_axon_ntff_profile_hook
    sys.modules["antenv.axon_hooks"] = mod
    antenv.axon_hooks = mod


def kernel(x, h1, h2, s1, s2, W, b):
    global _PROGRAM, LAST_RESULTS
    from concourse import bass_utils

    _ensure_ntff_hook()

    if _PROGRAM is None:
        _PROGRAM = _build_program()
    nc = _PROGRAM

    in_maps = _precompute_inputs(x, h1, h2, s1, s2, W, b)
    res = bass_utils.run_bass_kernel_spmd(nc, in_maps, core_ids=list(range(N_CORES)))
    LAST_RESULTS = res

    out = np.empty((B, N_CLASSES), np.float32)
    for m in range(N_CORES):
        c_lo, c_hi = _CLS_OFF[m], _CLS_OFF[m + 1]
        out[:, c_lo:c_hi] = res.results[m]["out"][:, : c_hi - c_lo]
    return out


# revision 43
# speedup vs baseline: 1.0052x; 1.0052x over previous
"""Trainium2 Bass kernel for nn_CustomClassifier_56307021251047 (compact bilinear pooling).

Math reformulation (validated against reference in numpy):
  count-sketch + rfft fold into dense matmuls with host-precomputed trig
  matrices:  A_s[b,f] = sum_i s_i x[b,i] e^{-2pi i h_i f / D}  ==  x @ C_s
  (C_s[i,f] built from (h_s, s_s) on host; the Nyquist bin f=D/2, which is
  real like bin 0, is packed into bin 0's imaginary slot so the spectrum is
  exactly NF = D/2 = 4096 bins = 8 cores x 512).
  Y = A_1 * A_2 (complex pointwise; bin 0 uses (re*re, im*im) instead).
  cbp = Yre @ ICre + Yim @ ICim  (inverse rfft as matmul, Nyquist row fixed).
  out = cbp @ W.T + b.

Sharding over 8 cores: spectrum dim f (4096 = 8 x 512) for stages 1-2,
AllReduce of the partial cbp^T [8192, 256], then classes (14951 -> 8 chunks)
for the Linear.
"""

import os
import sys

sys.path.insert(0, "/opt/trn_rl_repo")

import numpy as np

B, F_IN, D, N_CLASSES = 256, 2048, 8192, 14951
N_CORES = 8
NF = D // 2              # 4096 one-sided spectrum bins (Nyquist packed in bin0.im)
FC = NF // N_CORES       # 512 bins per core
FT = FC // 128           # 4 f-tiles per core
KX = F_IN // 128         # 16 k-tiles over input features
NT = D // 128            # 64 n-tiles of cbp
CPAD = 1872              # per-core padded class count (8*1872 >= 14951)
CW = [512, 512, 512, 336]       # class chunks within CPAD
C0 = [0, 512, 1024, 1536]
# class ranges per core: 7 x 1869 + 1868
_CLS_SIZES = [1869] * 7 + [1868]
_CLS_OFF = np.cumsum([0] + _CLS_SIZES).tolist()

# dtype knobs (env-overridable for experiments)
_DT_S1 = os.environ.get("K_DT_S1", "bfloat16")  # stage-1 matmul (cpack & xT)
_DT_S2 = os.environ.get("K_DT_S2", "bfloat16")  # stage-2 matmul (IC & Y)
_DT_W = os.environ.get("K_DT_W", "bfloat16")    # stage-3 matmul (W & cbp lhsT)


LAST_RESULTS = None  # BassKernelResults of the final launch, for test.py


def _np_dt(name):
    if name == "bfloat16":
        import ml_dtypes

        return ml_dtypes.bfloat16
    return np.dtype(name)


def _mybir_dt(name):
    import concourse.mybir as mybir

    return {"float32": mybir.dt.float32, "bfloat16": mybir.dt.bfloat16}[name]


def _build_program():
    import concourse.bacc as bacc
    import concourse.tile as tile
    import concourse.mybir as mybir

    f32 = mybir.dt.float32
    dt1 = _mybir_dt(_DT_S1)
    dt2 = _mybir_dt(_DT_S2)
    dtw = _mybir_dt(_DT_W)

    nc = bacc.Bacc("TRN2", target_bir_lowering=False, debug=False,
                   num_devices=N_CORES)

    xt = nc.dram_tensor("xt", [F_IN, B], dt1, kind="ExternalInput")
    cpack = nc.dram_tensor("cpack", [4, 128, 4, 4, FC], dt1, kind="ExternalInput")
    icpack = nc.dram_tensor("icpack", [8, 128, 64, 128], dt2, kind="ExternalInput")
    wpack = nc.dram_tensor("wpack", [NT + 1, 128, CPAD], dtw, kind="ExternalInput")
    fmask = nc.dram_tensor("fmask", [128, FT, 3], f32, kind="ExternalInput")
    out = nc.dram_tensor("out", [B, CPAD], f32, kind="ExternalOutput")

    rg = [list(range(N_CORES))]

    with tile.TileContext(nc) as tc:
        with (
            tc.tile_pool(name="const", bufs=1) as const,
            tc.tile_pool(name="dram", bufs=1, space="DRAM") as dram,
        ):
            # AllGather buffers: Y spectra (stage 1 -> 2) and cbp slabs (2 -> 3)
            ag1_ins = [
                dram.tile([128, FT, B], dt2, tag=f"ag1i{h}", name=f"ag1_in_{h}")
                for h in range(2)
            ]
            ag1_outs = [
                dram.tile([N_CORES, 128, FT, B], dt2, tag=f"ag1o{h}",
                          name=f"ag1_out_{h}", addr_space="Shared")
                for h in range(2)
            ]
            ag2_ins = [
                dram.tile([128, 4, B], dtw, tag=f"ag2i{j}", name=f"ag2_in_{j}")
                for j in range(2)
            ]
            ag2_outs = [
                dram.tile([N_CORES, 128, 4, B], dtw, tag=f"ag2o{j}",
                          name=f"ag2_out_{j}", addr_space="Shared")
                for j in range(2)
            ]

            fm = const.tile([128, FT, 3], f32)
            nc.sync.dma_start(out=fm[:], in_=fmask[:])

            # ncfw warmup: a tiny AllGather up front absorbs the first-call
            # collective latency while stage 1 runs.
            if os.environ.get("K_CCWARM", "1") == "1":
                agw_in = dram.tile([1, 64], f32, tag="agwi")
                agw_out = dram.tile([N_CORES, 64], f32, tag="agwo",
                                    addr_space="Shared")
                wtiny = const.tile([1, 64], f32, name="wtiny")
                nc.vector.memset(wtiny[:], 0.0)
                nc.sync.dma_start(out=agw_in.opt(), in_=wtiny[:])
                nc.gpsimd.collective_compute(
                    "AllGather", mybir.AluOpType.bypass, replica_groups=rg,
                    ins=[agw_in.opt()], outs=[agw_out.opt()],
                )

            # PE warmup: keep TensorE busy through the initial DMA ramp so
            # the HAM clock gate opens (1.2 -> 2.4 GHz) before stage 1.
            n_warm = int(os.environ.get("K_WARM", "48"))
            if n_warm:
                with tc.tile_pool(name="warm", bufs=1, space="PSUM") as warmp:
                    wsrc = const.tile([128, B], _mybir_dt(_DT_S1), name="wsrc")
                    nc.vector.memset(wsrc[:], 0.0)
                    wps = warmp.tile([128, B], f32, name="warm_ps")
                    for _ in range(n_warm):
                        nc.tensor.matmul(wps[:], wsrc[:, 0:128], wsrc[:],
                                         start=True, stop=True)
                    nc.vector.tensor_copy(out=wsrc[0:1, 0:8], in_=wps[0:1, 0:8])

            # ---------------- stage 1 + pointwise ----------------
            with (
                tc.tile_pool(name="xpool", bufs=1) as xpool,
                tc.tile_pool(name="apool", bufs=1) as apool,
                tc.tile_pool(name="ypool", bufs=1) as ypool,
                tc.tile_pool(name="cpool", bufs=1) as cpool,
                tc.tile_pool(name="tpool", bufs=1) as tpool,
            ):
                xt_sb = xpool.tile([128, KX, B], dt1)
                nc.sync.dma_start(
                    out=xt_sb[:], in_=xt.rearrange("(k p) b -> p k b", p=128)
                )
                # resident C matrices: [p, k, j, f] (8.4 MB bf16), 4 big DMAs
                cpk = cpool.tile([128, KX, 4, FC], dt1)
                for g in range(4):
                    nc.sync.dma_start(
                        out=cpk[:, 4 * g:4 * g + 4, :, :], in_=cpack[g]
                    )

                # stage 1: A_jT [FC, B] for j in (A1re, A1im, A2re, A2im),
                # halves grouped by f-tile so pointwise overlaps half 2.
                # a_sb tile index = j*FT + mt; y_sb index = rm*FT + ft
                a_sb = apool.tile([128, 4 * FT, B], f32)
                y_sb = ypool.tile([128, 2 * FT, B], dt2)
                with tc.tile_pool(name="ps1", bufs=1, space="PSUM") as ps1pool:
                    for h in range(2):  # f-tile halves: mt in {2h, 2h+1}
                        psums = [
                            ps1pool.tile([128, B], f32, tag=f"ps1_{j}_{mtl}",
                                         name=f"ps1_{h}_{j}_{mtl}")
                            for j in range(4)
                            for mtl in range(2)
                        ]
                        for k in range(KX):
                            for j in range(4):
                                for mtl in range(2):
                                    mt = 2 * h + mtl
                                    nc.tensor.matmul(
                                        psums[j * 2 + mtl][:],
                                        cpk[:, k, j, mt * 128:(mt + 1) * 128],
                                        xt_sb[:, k, :],
                                        start=(k == 0),
                                        stop=(k == KX - 1),
                                    )
                        for j in range(4):
                            for mtl in range(2):
                                mt = 2 * h + mtl
                                nc.vector.tensor_copy(
                                    out=a_sb[:, j * FT + mt, :],
                                    in_=psums[j * 2 + mtl][:],
                                )
                        # pointwise for ft in this half: Y = A1*A2 complex,
                        # with packed-bin fix via masks
                        for mtl in range(2):
                            ft = 2 * h + mtl
                            r1 = a_sb[:, 0 * FT + ft, :]
                            i1 = a_sb[:, 1 * FT + ft, :]
                            r2 = a_sb[:, 2 * FT + ft, :]
                            i2 = a_sb[:, 3 * FT + ft, :]
                            cm = fm[:, ft, 0:1]
                            ncm = fm[:, ft, 1:2]
                            mk = fm[:, ft, 2:3]
                            tt = tpool.tile([128, 4, B], f32, tag="ptmp",
                                            name=f"ptmp_{ft}")
                            T, U, V, W2 = (tt[:, i, :] for i in range(4))
                            nc.vector.tensor_mul(out=T, in0=i1, in1=i2)
                            nc.vector.tensor_mul(out=U, in0=r1, in1=r2)
                            # Yre = (T * ncm) + U
                            nc.vector.scalar_tensor_tensor(
                                out=y_sb[:, ft * 2, :], in0=T, scalar=ncm,
                                in1=U, op0=mybir.AluOpType.mult,
                                op1=mybir.AluOpType.add,
                            )
                            nc.vector.tensor_mul(out=V, in0=r1, in1=i2)
                            nc.vector.tensor_mul(out=W2, in0=i1, in1=r2)
                            nc.vector.tensor_add(out=V, in0=V, in1=W2)
                            nc.vector.tensor_scalar_mul(T, T, mk)
                            # Yim = (V * cm) + (T * mk)
                            nc.vector.scalar_tensor_tensor(
                                out=y_sb[:, ft * 2 + 1, :], in0=V, scalar=cm,
                                in1=T, op0=mybir.AluOpType.mult,
                                op1=mybir.AluOpType.add,
                            )
                        # ship this half's Y (tiles 4h..4h+4) and gather
                        nc.sync.dma_start(
                            out=ag1_ins[h].opt(),
                            in_=y_sb[:, 4 * h:4 * h + 4, :],
                        )
                        nc.gpsimd.collective_compute(
                            "AllGather", mybir.AluOpType.bypass,
                            replica_groups=rg,
                            ins=[ag1_ins[h].opt()], outs=[ag1_outs[h].opt()],
                        )

            # ---------------- stage 2 (n-sharded over cores) ----------------
            # cbpT[n_m, b] = sum over ALL f of IC[f, n_m] * YT[f, b],
            # n_m = this core's 1024-wide slab; kstep = c*8 + rm*4 + ftl
            with (
                tc.tile_pool(name="yfpool", bufs=1) as yfpool,
                tc.tile_pool(name="icpool", bufs=3) as icpool,
                tc.tile_pool(name="cbspool", bufs=2) as cbspool,
                tc.tile_pool(name="ps2", bufs=4, space="PSUM") as ps2pool,
            ):
                # y_full slot ks = h*32 + c*4 + (ftl2*2 + rm)
                y_full = yfpool.tile([128, 8 * N_CORES, B], dt2)
                for h in range(2):
                    for c in range(N_CORES):
                        nc.sync.dma_start(
                            out=y_full[:, h * 32 + c * 4:h * 32 + c * 4 + 4, :],
                            in_=ag1_outs[h][c],
                        )
                for nt in range(8):
                    ic_tile = icpool.tile([128, 64, 128], dt2, tag="ictile",
                                          name=f"ic_{nt}")
                    nc.sync.dma_start(out=ic_tile[:], in_=icpack[nt])
                    half = nt // 4
                    if nt % 4 == 0:
                        cbs = cbspool.tile([128, 4, B], dtw, tag="cbs",
                                           name=f"cbs_{half}")
                    ps = ps2pool.tile([128, B], f32, tag="ps2",
                                      name=f"ps2_{nt}")
                    for ks in range(64):
                        nc.tensor.matmul(
                            ps[:],
                            ic_tile[:, ks, :],
                            y_full[:, ks, :],
                            start=(ks == 0),
                            stop=(ks == 63),
                        )
                    nc.vector.tensor_copy(out=cbs[:, nt % 4, :], in_=ps[:])
                    if nt % 4 == 3:
                        nc.sync.dma_start(out=ag2_ins[half].opt(), in_=cbs[:])
                        nc.gpsimd.collective_compute(
                            "AllGather", mybir.AluOpType.bypass,
                            replica_groups=rg,
                            ins=[ag2_ins[half].opt()],
                            outs=[ag2_outs[half].opt()],
                        )

            # ---------------- stage 3 ----------------
            with (
                tc.tile_pool(name="cbpool", bufs=3) as cbpool,
                tc.tile_pool(name="wpool", bufs=6) as wpool,
                tc.tile_pool(name="opool", bufs=1) as opool,
                tc.tile_pool(name="ps3", bufs=1, space="PSUM") as ps3pool,
            ):
                kones = const.tile([128, B], dtw)
                nc.vector.memset(kones[:], 0.0)
                nc.vector.memset(kones[0:1, :], 1.0)

                psums3 = [
                    ps3pool.tile([128, CW[cc]], f32, tag=f"ps3_{cc}_{bh}",
                                 name=f"ps3_{cc}_{bh}")
                    for cc in range(4)
                    for bh in range(2)
                ]
                # bias step first (k = NT): runs before any AR data lands
                w_tile = wpool.tile([128, CPAD], dtw, tag="wt", name="wt_bias")
                nc.sync.dma_start(out=w_tile[:], in_=wpack[NT])
                for cc in range(4):
                    for bh in range(2):
                        nc.tensor.matmul(
                            psums3[cc * 2 + bh][:],
                            kones[:, bh * 128:(bh + 1) * 128],
                            w_tile[:, C0[cc]:C0[cc] + CW[cc]],
                            start=True,
                            stop=False,
                        )
                for j in range(2):
                    for c in range(N_CORES):
                        cb_tile = cbpool.tile([128, 4, B], dtw, tag="cbt",
                                              name=f"cbt_{j}_{c}")
                        nc.sync.dma_start(out=cb_tile[:], in_=ag2_outs[j][c])
                        for i in range(4):
                            k = 8 * c + 4 * j + i
                            w_tile = wpool.tile([128, CPAD], dtw, tag="wt",
                                                name=f"wt_{k}")
                            nc.sync.dma_start(out=w_tile[:], in_=wpack[k])
                            last = (j == 1 and c == N_CORES - 1 and i == 3)
                            for cc in range(4):
                                for bh in range(2):
                                    nc.tensor.matmul(
                                        psums3[cc * 2 + bh][:],
                                        cb_tile[:, i, bh * 128:(bh + 1) * 128],
                                        w_tile[:, C0[cc]:C0[cc] + CW[cc]],
                                        start=False,
                                        stop=last,
                                    )

                out_sb = opool.tile([128, 2, CPAD], f32)
                for cc in range(4):
                    for bh in range(2):
                        nc.vector.tensor_copy(
                            out=out_sb[:, bh, C0[cc]:C0[cc] + CW[cc]],
                            in_=psums3[cc * 2 + bh][:],
                        )
                nc.sync.dma_start(
                    out=out.rearrange("(h p) c -> p h c", p=128), in_=out_sb[:]
                )

    nc.compile()
    return nc


def _precompute_inputs(x, h1, h2, s1, s2, W, b):
    """Host-side packing of weights/constants into per-core input maps."""
    dt1 = _np_dt(_DT_S1)
    dt2 = _np_dt(_DT_S2)
    dtw = _np_dt(_DT_W)

    h1 = np.asarray(h1).astype(np.int64)
    h2 = np.asarray(h2).astype(np.int64)
    s1 = np.asarray(s1).astype(np.float64)
    s2 = np.asarray(s2).astype(np.float64)
    x = np.asarray(x, dtype=np.float32)
    W = np.asarray(W, dtype=np.float32)
    b = np.asarray(b, dtype=np.float32)

    xt_np = np.ascontiguousarray(x.T).astype(dt1)  # [F_IN, B]

    f = np.arange(NF, dtype=np.int64)

    def build_C(h, s):
        ang = 2.0 * np.pi * ((np.outer(h, f) % D).astype(np.float64)) / D
        Cre = s[:, None] * np.cos(ang)
        Cim = -s[:, None] * np.sin(ang)
        Cim[:, 0] = s * np.where(h % 2 == 0, 1.0, -1.0)  # Nyquist real part
        return Cre.astype(np.float32), Cim.astype(np.float32)

    C1re, C1im = build_C(h1, s1)
    C2re, C2im = build_C(h2, s2)

    n = np.arange(D, dtype=np.int64)
    ang = 2.0 * np.pi * ((np.outer(f, n) % D).astype(np.float64)) / D
    w_f = np.full((NF, 1), 2.0)
    w_f[0] = 1.0
    ICre = (w_f * np.cos(ang)) / D
    ICim = (-w_f * np.sin(ang)) / D
    ICim[0, :] = np.where(n % 2 == 0, 1.0, -1.0) / D  # packed Nyquist row
    ICre = ICre.astype(np.float32)
    ICim = ICim.astype(np.float32)

    in_maps = []
    for m in range(N_CORES):
        fsl = slice(m * FC, (m + 1) * FC)
        # cpack [4, 128, 4, 4, FC]: [g][p][kk][j][f], k = 4g + kk
        cp = np.empty((4, 128, 4, 4, FC), np.float32)
        for j_i, cmat in enumerate((C1re, C1im, C2re, C2im)):
            cj = cmat[:, fsl].reshape(4, 4, 128, FC)  # [g, kk, p, f]
            cp[:, :, :, j_i, :] = cj.transpose(0, 2, 1, 3)
        # icpack [8, 128, 64, 128]: [nt][f_in_tile][kstep][nn] where
        # kstep = h*32 + c*4 + (ftl%2)*2 + rm  (h = ftl//2),
        # f = c*512 + ftl*128 + p, n = m*1024 + nt*128 + nn (this core's slab)
        icp = np.empty((8, 128, 64, 128), np.float32)
        nsl = slice(m * 8 * 128, (m + 1) * 8 * 128)
        for rm, icm in enumerate((ICre, ICim)):
            # [c, ftl, p, nt, nn]
            blk = icm[:, nsl].reshape(N_CORES, FT, 128, 8, 128)
            for c in range(N_CORES):
                for ftl in range(FT):
                    ks = (ftl // 2) * 32 + c * 4 + (ftl % 2) * 2 + rm
                    icp[:, :, ks, :] = blk[c, ftl].transpose(1, 0, 2)
        # wpack [65, 128, CPAD]
        c_lo, c_hi = _CLS_OFF[m], _CLS_OFF[m + 1]
        sz = c_hi - c_lo
        wp = np.zeros((NT + 1, 128, CPAD), np.float32)
        wp[:NT, :, :sz] = W[c_lo:c_hi].T.reshape(NT, 128, sz)
        wp[NT, 0, :sz] = b[c_lo:c_hi]
        # fmask [128, FT, 3]: cm, negcm, mask
        fmsk = np.zeros((128, FT, 3), np.float32)
        fmsk[:, :, 0] = 1.0
        fmsk[:, :, 1] = -1.0
        if m == 0:
            fmsk[0, 0, 0] = 0.0
            fmsk[0, 0, 1] = 0.0
            fmsk[0, 0, 2] = 1.0
        in_maps.append(
            {
                "xt": xt_np,
                "cpack": cp.astype(dt1),
                "icpack": icp.astype(dt2),
                "wpack": wp.astype(dtw),
                "fmask": fmsk,
            }
        )
    return in_maps


_PROGRAM = None


def _ensure_ntff_hook():
    """bass_utils' axon trace path imports antenv.axon_hooks, which this
    container's antenv stub lacks. Provide it, backed by the ctypes NTFF
    profiler from trn_boot, so BASS_TRACE=1 profiling works."""
    import types

    try:
        import antenv.axon_hooks  # noqa: F401

        return
    except ImportError:
        pass
    try:
        import antenv
    except ImportError:
        return
    state = {"hook": None, "tried": False}

    def set_axon_ntff_profile_hook(h):
        state["hook"] = h
        state["tried"] = True

    def get_axon_ntff_profile_hook():
        if not state["tried"]:
            state["tried"] = True
            try:
                from trn_agent_boot.trn_boot import _ntff_profile_via_ctypes

                state["hook"] = _ntff_profile_via_ctypes("/opt/axon/libaxon_pjrt.so")
            except Exception:
                state["hook"] = None
        return state["hook"]

    mod = types.ModuleType("antenv.axon_hooks")
    mod.set_axon_ntff_profile_hook = set_axon_ntff_profile_hook
    mod.get_axon_ntff_profile_hook = get_axon_ntff_profile_hook
    sys.modules["antenv.axon_hooks"] = mod
    antenv.axon_hooks = mod


def kernel(x, h1, h2, s1, s2, W, b):
    global _PROGRAM, LAST_RESULTS
    from concourse import bass_utils

    _ensure_ntff_hook()

    if _PROGRAM is None:
        _PROGRAM = _build_program()
    nc = _PROGRAM

    in_maps = _precompute_inputs(x, h1, h2, s1, s2, W, b)
    res = bass_utils.run_bass_kernel_spmd(nc, in_maps, core_ids=list(range(N_CORES)))
    LAST_RESULTS = res

    out = np.empty((B, N_CLASSES), np.float32)
    for m in range(N_CORES):
        c_lo, c_hi = _CLS_OFF[m], _CLS_OFF[m + 1]
        out[:, c_lo:c_hi] = res.results[m]["out"][:, : c_hi - c_lo]
    return out


# revision 48
# speedup vs baseline: 1.0429x; 1.0375x over previous
"""Trainium2 Bass kernel for nn_CustomClassifier_56307021251047 (compact bilinear pooling).

Math reformulation (validated against reference in numpy):
  count-sketch + rfft fold into dense matmuls with host-precomputed trig
  matrices:  A_s[b,f] = sum_i s_i x[b,i] e^{-2pi i h_i f / D}  ==  x @ C_s
  (C_s[i,f] built from (h_s, s_s) on host; the Nyquist bin f=D/2, which is
  real like bin 0, is packed into bin 0's imaginary slot so the spectrum is
  exactly NF = D/2 = 4096 bins = 8 cores x 512).
  Y = A_1 * A_2 (complex pointwise; bin 0 uses (re*re, im*im) instead).
  cbp = Yre @ ICre + Yim @ ICim  (inverse rfft as matmul, Nyquist row fixed).
  out = cbp @ W.T + b.

Sharding over 8 cores: spectrum dim f (4096 = 8 x 512) for stages 1-2,
AllReduce of the partial cbp^T [8192, 256], then classes (14951 -> 8 chunks)
for the Linear.
"""

import os
import sys

sys.path.insert(0, "/opt/trn_rl_repo")

import numpy as np

B, F_IN, D, N_CLASSES = 256, 2048, 8192, 14951
N_CORES = 8
NF = D // 2              # 4096 one-sided spectrum bins (Nyquist packed in bin0.im)
FC = NF // N_CORES       # 512 bins per core
FT = FC // 128           # 4 f-tiles per core
KX = F_IN // 128         # 16 k-tiles over input features
NT = D // 128            # 64 n-tiles of cbp
CPAD = 1872              # per-core padded class count (8*1872 >= 14951)
CW = [512, 512, 512, 336]       # class chunks within CPAD
C0 = [0, 512, 1024, 1536]
# class ranges per core: 7 x 1869 + 1868
_CLS_SIZES = [1869] * 7 + [1868]
_CLS_OFF = np.cumsum([0] + _CLS_SIZES).tolist()

# dtype knobs (env-overridable for experiments)
_DT_S1 = os.environ.get("K_DT_S1", "bfloat16")  # stage-1 matmul (cpack & xT)
_DT_S2 = os.environ.get("K_DT_S2", "bfloat16")  # stage-2 matmul (IC & Y)
_DT_W = os.environ.get("K_DT_W", "bfloat16")    # stage-3 matmul (W & cbp lhsT)


LAST_RESULTS = None  # BassKernelResults of the final launch, for test.py


def _np_dt(name):
    if name == "bfloat16":
        import ml_dtypes

        return ml_dtypes.bfloat16
    return np.dtype(name)


def _mybir_dt(name):
    import concourse.mybir as mybir

    return {"float32": mybir.dt.float32, "bfloat16": mybir.dt.bfloat16}[name]


def _build_program():
    import concourse.bacc as bacc
    import concourse.tile as tile
    import concourse.mybir as mybir

    f32 = mybir.dt.float32
    dt1 = _mybir_dt(_DT_S1)
    dt2 = _mybir_dt(_DT_S2)
    dtw = _mybir_dt(_DT_W)

    nc = bacc.Bacc("TRN2", target_bir_lowering=False, debug=False,
                   num_devices=N_CORES)

    xt = nc.dram_tensor("xt", [F_IN, B], dt1, kind="ExternalInput")
    cpack = nc.dram_tensor("cpack", [4, 128, 4, 4, FC], dt1, kind="ExternalInput")
    icpack = nc.dram_tensor("icpack", [8, 128, 64, 128], dt2, kind="ExternalInput")
    wpack = nc.dram_tensor("wpack", [NT + 1, 128, CPAD], dtw, kind="ExternalInput")
    fmask = nc.dram_tensor("fmask", [128, FT, 3], f32, kind="ExternalInput")
    out = nc.dram_tensor("out", [B, CPAD], f32, kind="ExternalOutput")

    rg = [list(range(N_CORES))]

    with tile.TileContext(nc) as tc:
        with (
            tc.tile_pool(name="const", bufs=1) as const,
            tc.tile_pool(name="dram", bufs=1, space="DRAM") as dram,
        ):
            # AllGather buffers: Y spectra (stage 1 -> 2) and cbp slabs (2 -> 3)
            ag1_ins = [
                dram.tile([128, FT, B], dt2, tag=f"ag1i{h}", name=f"ag1_in_{h}")
                for h in range(2)
            ]
            ag1_outs = [
                dram.tile([N_CORES, 128, FT, B], dt2, tag=f"ag1o{h}",
                          name=f"ag1_out_{h}", addr_space="Shared")
                for h in range(2)
            ]
            ag2_ins = [
                dram.tile([128, 4, B], dtw, tag=f"ag2i{j}", name=f"ag2_in_{j}")
                for j in range(2)
            ]
            ag2_outs = [
                dram.tile([N_CORES, 128, 4, B], dtw, tag=f"ag2o{j}",
                          name=f"ag2_out_{j}", addr_space="Shared")
                for j in range(2)
            ]

            fm = const.tile([128, FT, 3], f32)
            nc.sync.dma_start(out=fm[:], in_=fmask[:])

            # ncfw warmup: a tiny AllGather up front absorbs the first-call
            # collective latency while stage 1 runs.
            if os.environ.get("K_CCWARM", "0") == "1":
                agw_in = dram.tile([1, 64], f32, tag="agwi")
                agw_out = dram.tile([N_CORES, 64], f32, tag="agwo",
                                    addr_space="Shared")
                wtiny = const.tile([1, 64], f32, name="wtiny")
                nc.vector.memset(wtiny[:], 0.0)
                nc.sync.dma_start(out=agw_in.opt(), in_=wtiny[:])
                nc.gpsimd.collective_compute(
                    "AllGather", mybir.AluOpType.bypass, replica_groups=rg,
                    ins=[agw_in.opt()], outs=[agw_out.opt()],
                )

            # PE warmup: keep TensorE busy through the initial DMA ramp so
            # the HAM clock gate opens (1.2 -> 2.4 GHz) before stage 1.
            n_warm = int(os.environ.get("K_WARM", "48"))
            if n_warm:
                with tc.tile_pool(name="warm", bufs=1, space="PSUM") as warmp:
                    wsrc = const.tile([128, B], _mybir_dt(_DT_S1), name="wsrc")
                    nc.vector.memset(wsrc[:], 0.0)
                    wps = warmp.tile([128, B], f32, name="warm_ps")
                    for _ in range(n_warm):
                        nc.tensor.matmul(wps[:], wsrc[:, 0:128], wsrc[:],
                                         start=True, stop=True)
                    nc.vector.tensor_copy(out=wsrc[0:1, 0:8], in_=wps[0:1, 0:8])

            # ---------------- stage 1 + pointwise ----------------
            with (
                tc.tile_pool(name="xpool", bufs=1) as xpool,
                tc.tile_pool(name="apool", bufs=1) as apool,
                tc.tile_pool(name="ypool", bufs=1) as ypool,
                tc.tile_pool(name="cpool", bufs=1) as cpool,
                tc.tile_pool(name="tpool", bufs=1) as tpool,
            ):
                xt_sb = xpool.tile([128, KX, B], dt1)
                nc.sync.dma_start(
                    out=xt_sb[:], in_=xt.rearrange("(k p) b -> p k b", p=128)
                )
                # resident C matrices: [p, k, j, f] (8.4 MB bf16), 4 big DMAs
                cpk = cpool.tile([128, KX, 4, FC], dt1)
                for g in range(4):
                    nc.sync.dma_start(
                        out=cpk[:, 4 * g:4 * g + 4, :, :], in_=cpack[g]
                    )

                # stage 1: A_jT [FC, B] for j in (A1re, A1im, A2re, A2im),
                # halves grouped by f-tile so pointwise overlaps half 2.
                # a_sb tile index = j*FT + mt; y_sb index = rm*FT + ft
                a_sb = apool.tile([128, 4 * FT, B], f32)
                y_sb = ypool.tile([128, 2 * FT, B], dt2)
                with tc.tile_pool(name="ps1", bufs=1, space="PSUM") as ps1pool:
                    for h in range(2):  # f-tile halves: mt in {2h, 2h+1}
                        psums = [
                            ps1pool.tile([128, B], f32, tag=f"ps1_{j}_{mtl}",
                                         name=f"ps1_{h}_{j}_{mtl}")
                            for j in range(4)
                            for mtl in range(2)
                        ]
                        for k in range(KX):
                            for j in range(4):
                                for mtl in range(2):
                                    mt = 2 * h + mtl
                                    nc.tensor.matmul(
                                        psums[j * 2 + mtl][:],
                                        cpk[:, k, j, mt * 128:(mt + 1) * 128],
                                        xt_sb[:, k, :],
                                        start=(k == 0),
                                        stop=(k == KX - 1),
                                    )
                        for j in range(4):
                            for mtl in range(2):
                                mt = 2 * h + mtl
                                nc.vector.tensor_copy(
                                    out=a_sb[:, j * FT + mt, :],
                                    in_=psums[j * 2 + mtl][:],
                                )
                        # pointwise for ft in this half: Y = A1*A2 complex,
                        # with packed-bin fix via masks
                        for mtl in range(2):
                            ft = 2 * h + mtl
                            r1 = a_sb[:, 0 * FT + ft, :]
                            i1 = a_sb[:, 1 * FT + ft, :]
                            r2 = a_sb[:, 2 * FT + ft, :]
                            i2 = a_sb[:, 3 * FT + ft, :]
                            cm = fm[:, ft, 0:1]
                            ncm = fm[:, ft, 1:2]
                            mk = fm[:, ft, 2:3]
                            tt = tpool.tile([128, 4, B], f32, tag="ptmp",
                                            name=f"ptmp_{ft}")
                            T, U, V, W2 = (tt[:, i, :] for i in range(4))
                            nc.vector.tensor_mul(out=T, in0=i1, in1=i2)
                            nc.vector.tensor_mul(out=U, in0=r1, in1=r2)
                            # Yre = (T * ncm) + U
                            nc.vector.scalar_tensor_tensor(
                                out=y_sb[:, ft * 2, :], in0=T, scalar=ncm,
                                in1=U, op0=mybir.AluOpType.mult,
                                op1=mybir.AluOpType.add,
                            )
                            nc.vector.tensor_mul(out=V, in0=r1, in1=i2)
                            nc.vector.tensor_mul(out=W2, in0=i1, in1=r2)
                            nc.vector.tensor_add(out=V, in0=V, in1=W2)
                            nc.vector.tensor_scalar_mul(T, T, mk)
                            # Yim = (V * cm) + (T * mk)
                            nc.vector.scalar_tensor_tensor(
                                out=y_sb[:, ft * 2 + 1, :], in0=V, scalar=cm,
                                in1=T, op0=mybir.AluOpType.mult,
                                op1=mybir.AluOpType.add,
                            )
                        # ship this half's Y (tiles 4h..4h+4) and gather
                        nc.gpsimd.dma_start(
                            out=ag1_ins[h].opt(),
                            in_=y_sb[:, 4 * h:4 * h + 4, :],
                        )
                        nc.gpsimd.collective_compute(
                            "AllGather", mybir.AluOpType.bypass,
                            replica_groups=rg,
                            ins=[ag1_ins[h].opt()], outs=[ag1_outs[h].opt()],
                        )

            # ---------------- stage 2 (n-sharded over cores) ----------------
            # cbpT[n_m, b] = sum over ALL f of IC[f, n_m] * YT[f, b],
            # n_m = this core's 1024-wide slab; kstep = c*8 + rm*4 + ftl
            with (
                tc.tile_pool(name="yfpool", bufs=1) as yfpool,
                tc.tile_pool(name="icpool", bufs=4) as icpool,
                tc.tile_pool(name="cbspool", bufs=2) as cbspool,
                tc.tile_pool(name="cbpool", bufs=3) as cbpool,
                tc.tile_pool(name="wpool", bufs=12) as wpool,
                tc.tile_pool(name="opool", bufs=1) as opool,
            ):
                # y_full slot ks = h*32 + c*4 + (ftl2*2 + rm).
                # Dependent loads go on gpsimd (SWDGE) so the sync queue
                # streams IC and W panels without blocking on AG waits.
                y_full = yfpool.tile([128, 8 * N_CORES, B], dt2)
                for h in range(2):
                    for c in range(N_CORES):
                        nc.scalar.dma_start(
                            out=y_full[:, h * 32 + c * 4:h * 32 + c * 4 + 4, :],
                            in_=ag1_outs[h][c],
                        )
                with tc.tile_pool(name="ps2", bufs=4, space="PSUM") as ps2pool:
                    for nt in range(8):
                        ic_tile = icpool.tile([128, 64, 128], dt2, tag="ictile",
                                              name=f"ic_{nt}")
                        nc.sync.dma_start(out=ic_tile[:], in_=icpack[nt])
                        half = nt // 4
                        if nt % 4 == 0:
                            cbs = cbspool.tile([128, 4, B], dtw, tag="cbs",
                                               name=f"cbs_{half}")
                        ps = ps2pool.tile([128, B], f32, tag="ps2",
                                          name=f"ps2_{nt}")
                        for ks in range(64):
                            nc.tensor.matmul(
                                ps[:],
                                ic_tile[:, ks, :],
                                y_full[:, ks, :],
                                start=(ks == 0),
                                stop=(ks == 63),
                            )
                        nc.vector.tensor_copy(out=cbs[:, nt % 4, :], in_=ps[:])
                        if nt % 4 == 3:
                            nc.gpsimd.dma_start(out=ag2_ins[half].opt(),
                                                in_=cbs[:])
                            nc.gpsimd.collective_compute(
                                "AllGather", mybir.AluOpType.bypass,
                                replica_groups=rg,
                                ins=[ag2_ins[half].opt()],
                                outs=[ag2_outs[half].opt()],
                            )

                # ---------------- stage 3 ----------------
                with tc.tile_pool(name="ps3", bufs=1, space="PSUM") as ps3pool:
                    kones = const.tile([128, B], dtw)
                    nc.vector.memset(kones[:], 0.0)
                    nc.vector.memset(kones[0:1, :], 1.0)

                    psums3 = [
                        ps3pool.tile([128, CW[cc]], f32, tag=f"ps3_{cc}_{bh}",
                                     name=f"ps3_{cc}_{bh}")
                        for cc in range(4)
                        for bh in range(2)
                    ]
                    # bias step first (k = NT): runs before any AG data lands
                    w_tile = wpool.tile([128, CPAD], dtw, tag="wt",
                                        name="wt_bias")
                    nc.sync.dma_start(out=w_tile[:], in_=wpack[NT])
                    for cc in range(4):
                        for bh in range(2):
                            nc.tensor.matmul(
                                psums3[cc * 2 + bh][:],
                                kones[:, bh * 128:(bh + 1) * 128],
                                w_tile[:, C0[cc]:C0[cc] + CW[cc]],
                                start=True,
                                stop=False,
                            )
                    for j in range(2):
                        for c in range(N_CORES):
                            cb_tile = cbpool.tile([128, 4, B], dtw, tag="cbt",
                                                  name=f"cbt_{j}_{c}")
                            nc.scalar.dma_start(out=cb_tile[:],
                                                in_=ag2_outs[j][c])
                            for i in range(4):
                                k = 8 * c + 4 * j + i
                                w_tile = wpool.tile([128, CPAD], dtw, tag="wt",
                                                    name=f"wt_{k}")
                                nc.sync.dma_start(out=w_tile[:], in_=wpack[k])
                                last = (j == 1 and c == N_CORES - 1 and i == 3)
                                for cc in range(4):
                                    for bh in range(2):
                                        nc.tensor.matmul(
                                            psums3[cc * 2 + bh][:],
                                            cb_tile[:, i,
                                                    bh * 128:(bh + 1) * 128],
                                            w_tile[:, C0[cc]:C0[cc] + CW[cc]],
                                            start=False,
                                            stop=last,
                                        )

                    out_sb = opool.tile([128, 2, CPAD], f32)
                    for cc in range(4):
                        for bh in range(2):
                            nc.vector.tensor_copy(
                                out=out_sb[:, bh, C0[cc]:C0[cc] + CW[cc]],
                                in_=psums3[cc * 2 + bh][:],
                            )
                    nc.sync.dma_start(
                        out=out.rearrange("(h p) c -> p h c", p=128),
                        in_=out_sb[:],
                    )

    nc.compile()
    return nc


def _precompute_inputs(x, h1, h2, s1, s2, W, b):
    """Host-side packing of weights/constants into per-core input maps."""
    dt1 = _np_dt(_DT_S1)
    dt2 = _np_dt(_DT_S2)
    dtw = _np_dt(_DT_W)

    h1 = np.asarray(h1).astype(np.int64)
    h2 = np.asarray(h2).astype(np.int64)
    s1 = np.asarray(s1).astype(np.float64)
    s2 = np.asarray(s2).astype(np.float64)
    x = np.asarray(x, dtype=np.float32)
    W = np.asarray(W, dtype=np.float32)
    b = np.asarray(b, dtype=np.float32)

    xt_np = np.ascontiguousarray(x.T).astype(dt1)  # [F_IN, B]

    f = np.arange(NF, dtype=np.int64)

    def build_C(h, s):
        ang = 2.0 * np.pi * ((np.outer(h, f) % D).astype(np.float64)) / D
        Cre = s[:, None] * np.cos(ang)
        Cim = -s[:, None] * np.sin(ang)
        Cim[:, 0] = s * np.where(h % 2 == 0, 1.0, -1.0)  # Nyquist real part
        return Cre.astype(np.float32), Cim.astype(np.float32)

    C1re, C1im = build_C(h1, s1)
    C2re, C2im = build_C(h2, s2)

    n = np.arange(D, dtype=np.int64)
    ang = 2.0 * np.pi * ((np.outer(f, n) % D).astype(np.float64)) / D
    w_f = np.full((NF, 1), 2.0)
    w_f[0] = 1.0
    ICre = (w_f * np.cos(ang)) / D
    ICim = (-w_f * np.sin(ang)) / D
    ICim[0, :] = np.where(n % 2 == 0, 1.0, -1.0) / D  # packed Nyquist row
    ICre = ICre.astype(np.float32)
    ICim = ICim.astype(np.float32)

    in_maps = []
    for m in range(N_CORES):
        fsl = slice(m * FC, (m + 1) * FC)
        # cpack [4, 128, 4, 4, FC]: [g][p][kk][j][f], k = 4g + kk
        cp = np.empty((4, 128, 4, 4, FC), np.float32)
        for j_i, cmat in enumerate((C1re, C1im, C2re, C2im)):
            cj = cmat[:, fsl].reshape(4, 4, 128, FC)  # [g, kk, p, f]
            cp[:, :, :, j_i, :] = cj.transpose(0, 2, 1, 3)
        # icpack [8, 128, 64, 128]: [nt][f_in_tile][kstep][nn] where
        # kstep = h*32 + c*4 + (ftl%2)*2 + rm  (h = ftl//2),
        # f = c*512 + ftl*128 + p, n = m*1024 + nt*128 + nn (this core's slab)
        icp = np.empty((8, 128, 64, 128), np.float32)
        nsl = slice(m * 8 * 128, (m + 1) * 8 * 128)
        for rm, icm in enumerate((ICre, ICim)):
            # [c, ftl, p, nt, nn]
            blk = icm[:, nsl].reshape(N_CORES, FT, 128, 8, 128)
            for c in range(N_CORES):
                for ftl in range(FT):
                    ks = (ftl // 2) * 32 + c * 4 + (ftl % 2) * 2 + rm
                    icp[:, :, ks, :] = blk[c, ftl].transpose(1, 0, 2)
        # wpack [65, 128, CPAD]
        c_lo, c_hi = _CLS_OFF[m], _CLS_OFF[m + 1]
        sz = c_hi - c_lo
        wp = np.zeros((NT + 1, 128, CPAD), np.float32)
        wp[:NT, :, :sz] = W[c_lo:c_hi].T.reshape(NT, 128, sz)
        wp[NT, 0, :sz] = b[c_lo:c_hi]
        # fmask [128, FT, 3]: cm, negcm, mask
        fmsk = np.zeros((128, FT, 3), np.float32)
        fmsk[:, :, 0] = 1.0
        fmsk[:, :, 1] = -1.0
        if m == 0:
            fmsk[0, 0, 0] = 0.0
            fmsk[0, 0, 1] = 0.0
            fmsk[0, 0, 2] = 1.0
        in_maps.append(
            {
                "xt": xt_np,
                "cpack": cp.astype(dt1),
                "icpack": icp.astype(dt2),
                "wpack": wp.astype(dtw),
                "fmask": fmsk,
            }
        )
    return in_maps


_PROGRAM = None


def _ensure_ntff_hook():
    """bass_utils' axon trace path imports antenv.axon_hooks, which this
    container's antenv stub lacks. Provide it, backed by the ctypes NTFF
    profiler from trn_boot, so BASS_TRACE=1 profiling works."""
    import types

    try:
        import antenv.axon_hooks  # noqa: F401

        return
    except ImportError:
        pass
    try:
        import antenv
    except ImportError:
        return
    state = {"hook": None, "tried": False}

    def set_axon_ntff_profile_hook(h):
        state["hook"] = h
        state["tried"] = True

    def get_axon_ntff_profile_hook():
        if not state["tried"]:
            state["tried"] = True
            try:
                from trn_agent_boot.trn_boot import _ntff_profile_via_ctypes

                state["hook"] = _ntff_profile_via_ctypes("/opt/axon/libaxon_pjrt.so")
            except Exception:
                state["hook"] = None
        return state["hook"]

    mod = types.ModuleType("antenv.axon_hooks")
    mod.set_axon_ntff_profile_hook = set_axon_ntff_profile_hook
    mod.get_axon_ntff_profile_hook = get_axon_ntff_profile_hook
    sys.modules["antenv.axon_hooks"] = mod
    antenv.axon_hooks = mod


def kernel(x, h1, h2, s1, s2, W, b):
    global _PROGRAM, LAST_RESULTS
    from concourse import bass_utils

    _ensure_ntff_hook()

    if _PROGRAM is None:
        _PROGRAM = _build_program()
    nc = _PROGRAM

    in_maps = _precompute_inputs(x, h1, h2, s1, s2, W, b)
    res = bass_utils.run_bass_kernel_spmd(nc, in_maps, core_ids=list(range(N_CORES)))
    LAST_RESULTS = res

    out = np.empty((B, N_CLASSES), np.float32)
    for m in range(N_CORES):
        c_lo, c_hi = _CLS_OFF[m], _CLS_OFF[m + 1]
        out[:, c_lo:c_hi] = res.results[m]["out"][:, : c_hi - c_lo]
    return out
